# revision 1
# baseline (speedup 1.0000x reference)
import functools

import jax
import jax.numpy as jnp
import numpy as np

# Model dims (hardcoded per problem spec nn_DAT_68805376082211)
B, L, CIN, H, TD, LOUT = 4096, 27, 64, 256, 64, 3
NCORES = 8


def _layernorm(x, g, b, eps=1e-5):
    m = x.mean(-1, keepdims=True)
    v = ((x - m) ** 2).mean(-1, keepdims=True)
    return (x - m) / jnp.sqrt(v + eps) * g + b


def _forward(x, target, ln_g, ln_b, Wv, bv, W0, b0, W1, b1, W2, b2, Wh, bh,
             Wt1, bt1, Wt2, bt2, Wo, bo):
    # x: [Bs, L, CIN], target: [Bs, TD] for this shard
    Bs = x.shape[0]
    v = _layernorm(x, ln_g, ln_b)                              # [Bs, L, CIN]
    vl = jax.nn.relu(jnp.einsum('blc,ch->blh', v, Wv) + bv)    # [Bs, L, 3H]
    V_ = vl.reshape(Bs, L, 3, H).transpose(0, 2, 1, 3)         # [Bs, 3, L, H]
    V0, V1, V2 = V_[:, 0], V_[:, 1], V_[:, 2]                  # [Bs, L, H]

    # head 0: pointwise
    sk0 = jax.nn.relu(jnp.einsum('blh,ho->blo', V0, W0) + b0)  # [Bs, L, LOUT]
    sk0 = sk0.transpose(0, 2, 1)                               # [Bs, LOUT, L]

    # head 1: circular conv k=3 dil=1 -> Y_k = V1 @ W1[:,:,k]; sk1[o,t] = sum_k Y_k[(t+k-1)%L, o]
    Y = jnp.einsum('blh,ohk->bklo', V1, W1)                    # [Bs, 3, L, LOUT]
    sk1 = (jnp.roll(Y[:, 0], 1, axis=1) + Y[:, 1] + jnp.roll(Y[:, 2], -1, axis=1))
    sk1 = jax.nn.relu(sk1 + b1[None, None, :]).transpose(0, 2, 1)  # [Bs, LOUT, L]

    # head 2: dil=2
    Z = jnp.einsum('blh,ohk->bklo', V2, W2)
    sk2 = (jnp.roll(Z[:, 0], 2, axis=1) + Z[:, 1] + jnp.roll(Z[:, 2], -2, axis=1))
    sk2 = jax.nn.relu(sk2 + b2[None, None, :]).transpose(0, 2, 1)

    sk = jnp.stack([sk0, sk1, sk2], 1)                         # [Bs, 3, LOUT, L]
    heads = jnp.einsum('bhol,bhld->bhod', sk, V_)              # [Bs, 3, LOUT, H]
    g = jnp.einsum('bhod,h->bod', heads, Wh) + bh              # [Bs, LOUT, H]

    ta = jax.nn.relu(target @ Wt1 + bt1)
    ta = jax.nn.relu(ta @ Wt2 + bt2)                           # [Bs, H]
    g = g * ta[:, None, :]

    out1 = g.mean(1)                                           # [Bs, H]
    out = jax.nn.relu(g.reshape(Bs, -1) @ Wo + bo) + out1      # [Bs, H]
    return out


_PMAPPED = None


def _get_pmapped():
    global _PMAPPED
    if _PMAPPED is None:
        _PMAPPED = jax.pmap(_forward, axis_name='i',
                            in_axes=(0, 0) + (None,) * 18)
    return _PMAPPED


def kernel(**inputs):
    x = np.asarray(inputs['x'], dtype=np.float32)
    target = np.asarray(inputs['target'], dtype=np.float32)
    params = [np.asarray(inputs[k], dtype=np.float32) for k in
              ('ln_g', 'ln_b', 'Wv', 'bv', 'W0', 'b0', 'W1', 'b1', 'W2', 'b2',
               'Wh', 'bh', 'Wt1', 'bt1', 'Wt2', 'bt2', 'Wo', 'bo')]
    xs = x.reshape(NCORES, B // NCORES, L, CIN)
    ts = target.reshape(NCORES, B // NCORES, TD)
    fn = _get_pmapped()
    out = fn(xs, ts, *params)
    return np.asarray(out).reshape(B, H)
